# revision 41
# baseline (speedup 1.0000x reference)
# Multi-headed attention (B=8, S=1024, D=1024, H=16) on 8 TRN2 NeuronCores.
# Strategy: pure batch data-parallel (one batch element per core, no
# collectives), all matmuls bf16 with fp32 PSUM accumulation. ~192us HW
# exec (staged baseline: 197us; original naive kernel: 430us).
#
# Structure: a DMA-paced front (warmup + all K projections + the first
# four Q halves, consuming exactly at the 2-ring DMA delivery rate), then
# a software-pipelined attention phase over (query-half, head-pair, key-
# chunk) with scores emitted 5 positions ahead of PV so the scalar
# engine's exp stream starts ~45us in and stays fed; ALL V-projection
# work drains just-in-time from deadline-ordered filler queues inside the
# attention loop. A staged output-projection tail finishes the run.
#
# Key optimizations:
#   - masked key positions are dropped on the host: key/value are gathered
#     to the unmasked positions (padded to a multiple of 128; an exp bias
#     of -30000 zeroes the pads exactly, matching the reference's -1e9
#     mask). The program is compiled per padded chunk count (nkc), cached.
#   - weights are pre-banded on the host so every DMA is a plain
#     contiguous [128, N] block transfer, issued alternately on the two
#     HWDGE queues (SP + Activation) for ~2x front delivery bandwidth.
#   - V tiles carry a per-head [ones | 63 dead | 64 dims] 128-wide group:
#     the PV matmul then lands the softmax denominator on PSUM partition 0
#     (the extra stationary columns are free - matmul cost is set by the
#     moving free dim). The whole PV tile is evacuated to SBUF in one DVE
#     op (cost depends only on free size), freeing the PSUM ring; the
#     reciprocal/broadcast/multiply normalize chain runs out of SBUF off
#     the critical path.
#   - all remaining projection work (Q quarters, late V chunks, and the
#     phase-2 O-projection of query blocks 0-3 in 4-matmul sub-groups)
#     drains from deadline-ordered filler queues inside the attention
#     loop, keeping the PE at full clock (HAM) while exp runs.
#   - the O-projection tail pre-accumulates pairs 0-4 for query blocks
#     4-7 on the then-idle pv/st PSUM rings while the final normalize
#     chains drain, leaving only short finishers after the last pair.
#   - the output bias (bv @ Wo + bo) is added on-device during the output
#     projection evacuation; output is written back bf16 to halve the
#     final writeback DMA (rel err stays ~5e-3, well under the 2e-2 gate).
import math
import sys

sys.path.insert(0, "/opt/trn_rl_repo")

from contextlib import ExitStack

import ml_dtypes
import numpy as np

import concourse.bass as bass
import concourse.mybir as mybir
from concourse import bacc
from concourse import tile
from concourse.bass_utils import run_bass_kernel_spmd

dt = mybir.dt
AF = mybir.ActivationFunctionType

B, S, D, H, DK = 8, 1024, 1024, 16, 64
P = 128
NCH = D // P  # 8 chunks of 128 along the 1024-sized dims
NPAIR = H // 2  # 8 head pairs
NEGB = -30000.0  # exp underflows to exactly 0.0, matching the -1e9 masking

_NC_CACHE = {}


def build_nc(nkc: int):
    SK = nkc * P  # gathered+padded key length
    SK2 = SK // 2
    lean = nkc >= 7  # dense-mask fallback: shallower stream buffers
    ET_BUFS = 3 if lean else 7
    PC_BUFS = 2 if lean else 3
    OB_BUFS = 2 if lean else 3
    CH_BUFS = 1 if lean else 2
    nc = bacc.Bacc()
    qT = nc.dram_tensor("qT", [D, S], dt.bfloat16, kind="ExternalInput")
    kTg = nc.dram_tensor("kTg", [D, SK], dt.bfloat16, kind="ExternalInput")
    vgb = nc.dram_tensor("vgb", [SK, D], dt.bfloat16, kind="ExternalInput")
    wqb = nc.dram_tensor("wqb", [D, D], dt.bfloat16, kind="ExternalInput")
    wkb = nc.dram_tensor("wkb", [D, D], dt.bfloat16, kind="ExternalInput")
    wv = nc.dram_tensor("wv", [D, D], dt.bfloat16, kind="ExternalInput")
    wo = nc.dram_tensor("wo", [D, D], dt.bfloat16, kind="ExternalInput")
    bq = nc.dram_tensor("bq", [P, NCH], dt.float32, kind="ExternalInput")
    bk = nc.dram_tensor("bk", [P, NCH], dt.float32, kind="ExternalInput")
    msk = nc.dram_tensor("msk", [P, nkc], dt.float32, kind="ExternalInput")
    bo = nc.dram_tensor("bo", [1, D], dt.float32, kind="ExternalInput")
    out = nc.dram_tensor("out", [S, D], dt.bfloat16, kind="ExternalOutput")

    with tile.TileContext(nc) as tc, ExitStack() as ctx:
        big = ctx.enter_context(tc.tile_pool(name="big", bufs=NCH))
        vp = ctx.enter_context(tc.tile_pool(name="vp", bufs=nkc))
        strm = ctx.enter_context(tc.tile_pool(name="strm", bufs=4))
        one = ctx.enter_context(tc.tile_pool(name="one", bufs=1))
        psp = ctx.enter_context(tc.tile_pool(name="psp", bufs=2, space="PSUM"))

        # ---- DMA emission in consumption order -------------------------
        # alternate the two HWDGE queues (SP + Activation) so the front
        # loads stream on two rings in parallel
        _dq = [0]

        def dma_load(dst, srcv):
            eng = nc.sync if _dq[0] % 2 == 0 else nc.scalar
            _dq[0] += 1
            eng.dma_start(dst, srcv)

        # PE warmup on a zeroed scratch tile: keeps the HAM activity window
        # busy while the first DMAs land so real work starts at 2.4 GHz.
        scr = one.tile([P, 512], dt.bfloat16, tag="scr")
        nc.gpsimd.memset(scr[:], 0.0)
        wps = psp.tile([P, 512], dt.float32, tag="proj")
        for _ in range(80):
            nc.tensor.matmul(
                wps[:, 0:P], scr[:, 0:P], scr[:, 512 - P : 512], start=True, stop=True
            )

        wkb_sb = [None] * NPAIR

        def load_wkb(p):
            t = big.tile([P, D], dt.bfloat16, tag="wkb")
            dma_load(t[:], wkb[p * P : (p + 1) * P, :])
            wkb_sb[p] = t

        load_wkb(0)
        xk = []
        for di in range(NCH):
            t = big.tile([P, SK], dt.bfloat16, tag="xk")
            dma_load(t[:], kTg[di * P : (di + 1) * P, :])
            xk.append(t)
        for p in range(1, NPAIR):
            load_wkb(p)

        # small constants
        msk_sb = one.tile([P, nkc], dt.float32, tag="msk")
        nc.sync.dma_start(msk_sb[:], msk[:])
        bq_sb = one.tile([P, NCH], dt.float32, tag="bq")
        nc.sync.dma_start(bq_sb[:], bq[:])
        bk_sb = one.tile([P, NCH], dt.float32, tag="bk")
        nc.sync.dma_start(bk_sb[:], bk[:])
        bo_row = one.tile([1, D], dt.float32, tag="bo_row")
        nc.sync.dma_start(bo_row[:], bo[:])

        # warm the ACT exp table while DMAs stream
        warm = one.tile([1, nkc], dt.float32, tag="warm")
        nc.scalar.activation(warm[:], msk_sb[0:1, :], AF.Exp, bias=0.0, scale=1.0)

        # output-bias row broadcast to all partitions
        bo_sb = one.tile([P, D], dt.float32, tag="bo_sb")
        nc.gpsimd.partition_broadcast(bo_sb[:], bo_row[:])

        # remaining loads, in consumption order
        wqb_sb = [None] * NPAIR

        def load_wqb(p):
            t = big.tile([P, D], dt.bfloat16, tag="wqb")
            dma_load(t[:], wqb[p * P : (p + 1) * P, :])
            wqb_sb[p] = t

        vgb_sb = [None] * nkc

        def load_vgb(kc):
            t = vp.tile([P, D], dt.bfloat16, tag="vgb")
            dma_load(t[:], vgb[kc * P : (kc + 1) * P, :])
            vgb_sb[kc] = t

        # Aggressive schedule (the common nkc==5 case): the front carries
        # only the K projections + four Q halves (exactly the DMA ramp);
        # ALL V-projection work drains as attention fillers, so the exp
        # stream starts ~13us earlier.
        AGG = nkc == 5
        NVF = 0 if AGG else (nkc if nkc >= 6 else min(3, nkc))
        nq_front = 4 if AGG else 2
        for p in range(nq_front):
            load_wqb(p)
        xq = []
        for di in range(NCH):
            t = big.tile([P, S], dt.bfloat16, tag="xq")
            dma_load(t[:], qT[di * P : (di + 1) * P, :])
            xq.append(t)
        wv_sb = []
        for di in range(NCH):
            t = big.tile([P, D], dt.bfloat16, tag="wv")
            dma_load(t[:], wv[di * P : (di + 1) * P, :])
            wv_sb.append(t)
        for kc in range(nkc):
            load_vgb(kc)
        for p in range(nq_front, NPAIR):
            load_wqb(p)
        wo_sb = []
        for pc in range(NCH):
            t = big.tile([P, D], dt.bfloat16, tag="wo")
            dma_load(t[:], wo[pc * P : (pc + 1) * P, :])
            wo_sb.append(t)

        # ---- work-unit emitters ---------------------------------------
        kt_t = [None] * NPAIR

        def emit_kt_half(p, half):
            # K projection of pair p, key half `half`, [d, s_k] layout
            if kt_t[p] is None:
                kt_t[p] = big.tile([P, SK], dt.bfloat16, tag="kt", name=f"kt{p}")
            hs = slice(half * SK2, (half + 1) * SK2)
            ps = psp.tile([P, 512], dt.float32, tag="proj", name=f"kt_ps{p}_{half}")
            for di in range(NCH):
                nc.tensor.matmul(
                    ps[:, 0:SK2],
                    wkb_sb[p][:, di * P : (di + 1) * P],
                    xk[di][:, hs],
                    start=(di == 0),
                    stop=(di == NCH - 1),
                )
            nc.vector.tensor_scalar_add(
                kt_t[p][:, hs], ps[:, 0:SK2], bk_sb[:, p : p + 1]
            )

        # V tiles: per head a 128-wide group [ones | 63 dead | 64 dims] so
        # the PV output puts the softmax denominator on PSUM partition 0
        # (readable in place by reciprocal_approx_fast) and the dims at
        # partitions 64..127 (PSUM partition ranges cannot straddle the 64
        # line). The extra M is free: matmul cost is set by the moving free
        # dim N, not M.
        VW = P
        vv_t = [None] * nkc

        v_ps = {}

        def emit_vv_sub(kc, hf, sub):
            # V projection chunk kc, head-half hf, contraction sub-range
            # (di 0-3 / 4-7); the psum evacuation rides on sub 1
            if vv_t[kc] is None:
                t = vp.tile([P, H * VW], dt.bfloat16, tag="vv", name=f"vv{kc}")
                vv_t[kc] = t
                nc.gpsimd.memset(t[:], 1.0)
            t = vv_t[kc]
            hs = slice(hf * 512, (hf + 1) * 512)
            if sub == 0:
                v_ps[(kc, hf)] = psp.tile(
                    [P, 512], dt.float32, tag="proj", name=f"v_ps{kc}_{hf}"
                )
            ps = v_ps[(kc, hf)]
            for di in range(4 * sub, 4 * sub + 4):
                nc.tensor.matmul(
                    ps[:],
                    vgb_sb[kc][:, di * P : (di + 1) * P],
                    wv_sb[di][:, hs],
                    start=(di == 0),
                    stop=(di == NCH - 1),
                )
            if sub == 1:
                del v_ps[(kc, hf)]
                dst = t[:, hf * 8 * VW : (hf + 1) * 8 * VW].rearrange(
                    "p (h e) -> p h e", e=VW
                )[:, :, 64 : 64 + DK]
                srcv = ps[:].rearrange("p (h e) -> p h e", e=DK)
                nc.vector.tensor_copy(dst, srcv)

        def emit_vv(kc, half=None):
            halves = (0, 1) if half is None else (half,)
            for hf in halves:
                emit_vv_sub(kc, hf, 0)
                emit_vv_sub(kc, hf, 1)

        qt_t = [None] * NPAIR

        def emit_qt_quarter(p, quarter):
            if qt_t[p] is None:
                qt_t[p] = big.tile([P, S], dt.bfloat16, tag="qt", name=f"qt{p}")
            qs = slice(quarter * 256, (quarter + 1) * 256)
            ps = psp.tile([P, 512], dt.float32, tag="proj", name=f"qt_ps{p}_{quarter}")
            for di in range(NCH):
                nc.tensor.matmul(
                    ps[:, 0:256],
                    wqb_sb[p][:, di * P : (di + 1) * P],
                    xq[di][:, qs],
                    start=(di == 0),
                    stop=(di == NCH - 1),
                )
            nc.vector.tensor_scalar_add(
                qt_t[p][:, qs], ps[:, 0:256], bq_sb[:, p : p + 1]
            )

        def emit_qt_half(p, hq):
            emit_qt_quarter(p, 2 * hq)
            emit_qt_quarter(p, 2 * hq + 1)

        ct_t = [None] * NPAIR
        o_ps = {}

        def emit_oproj_pre(qc, half, npc, ps=None):
            # output projection for query block qc, D-half `half`: partial
            # accumulation over pairs 0..npc-1 (their ct rows are ready
            # before the last pair's)
            hs = slice(half * 512, (half + 1) * 512)
            if ps is None:
                ps = psp.tile(
                    [P, 512], dt.float32, tag="proj", name=f"o_ps{qc}_{half}"
                )
            o_ps[(qc, half)] = ps
            for pc in range(npc):
                nc.tensor.matmul(
                    ps[:],
                    ct_t[pc][:, qc * P : (qc + 1) * P],
                    wo_sb[pc][:, hs],
                    start=(pc == 0),
                    stop=False,
                )

        def emit_oproj_fin(qc, half, npc):
            # remaining pairs + bias + writeback
            hs = slice(half * 512, (half + 1) * 512)
            ps = o_ps.pop((qc, half))
            for pc in range(npc, NCH):
                nc.tensor.matmul(
                    ps[:],
                    ct_t[pc][:, qc * P : (qc + 1) * P],
                    wo_sb[pc][:, hs],
                    start=False,
                    stop=(pc == NCH - 1),
                )
            ob = strm.tile([P, 512], dt.bfloat16, tag="ob", bufs=OB_BUFS)
            nc.vector.tensor_add(ob[:], ps[:], bo_sb[:, hs])
            nc.sync.dma_start(out[qc * P : (qc + 1) * P, hs], ob[:])

        def emit_oproj_sub(qc, half, sub):
            hs = slice(half * 512, (half + 1) * 512)
            if sub == 0:
                o_ps[(qc, half)] = psp.tile(
                    [P, 512], dt.float32, tag="proj", name=f"o_ps{qc}_{half}"
                )
            ps = o_ps[(qc, half)]
            for pc in range(4 * sub, 4 * sub + 4):
                nc.tensor.matmul(
                    ps[:],
                    ct_t[pc][:, qc * P : (qc + 1) * P],
                    wo_sb[pc][:, hs],
                    start=(pc == 0),
                    stop=(pc == NCH - 1),
                )
            if sub == 1:
                del o_ps[(qc, half)]
                ob = strm.tile([P, 512], dt.bfloat16, tag="ob", bufs=OB_BUFS)
                nc.vector.tensor_add(ob[:], ps[:], bo_sb[:, hs])
                nc.sync.dma_start(out[qc * P : (qc + 1) * P, hs], ob[:])

        def emit_oproj(qc, half):
            emit_oproj_sub(qc, half, 0)
            emit_oproj_sub(qc, half, 1)

        def emit_dummies(n, name):
            dps = psp.tile([P, 512], dt.float32, tag="proj", name=name)
            for _ in range(n):
                nc.tensor.matmul(
                    dps[:, 0:P],
                    scr[:, 0:P],
                    scr[:, 512 - P : 512],
                    start=True,
                    stop=True,
                )

        # ---- filler queues --------------------------------------------
        # Attention is ACT(exp)-bound: each (pair, query-half) unit costs
        # ~5.7us of exp on the scalar engine vs ~3.2us of scores+PV on the
        # PE. The leftover PE capacity runs "filler" units popped from a
        # queue: remaining V chunks, Q projection quarters, and in phase 2
        # the O projection of query blocks 0-3 (whose ct rows completed in
        # phase 1). Units are small (~0.9-1.8us) so the 2-deep st ring can
        # absorb the delay they add before the next scores issue.
        f_q = [[], []]
        if AGG:
            # deadline-ordered: all V chunk halves (heads-half 0 feeds
            # pairs 0-3 almost immediately, half 1 feeds pair 4+), the
            # remaining Q-half-0 quarters, then the first Q-half-1s.
            # Units 0-1 pop 5 slots (kc 0-4) BEFORE the trailing PV so
            # each V chunk lands just ahead of its first PV use (LA=5).
            for kc in range(nkc):
                f_q[0] += [("vs", kc, 0, 0), ("vs", kc, 0, 1)]
            f_q[0] += [("q", 4, 0), ("q", 4, 1), ("q", 5, 0), ("q", 5, 1)]
            for kc in range(nkc):
                f_q[0] += [("vs", kc, 1, 0), ("vs", kc, 1, 1)]
            f_q[0] += [
                ("q", 6, 0), ("q", 6, 1), ("q", 7, 0), ("q", 7, 1),
                ("q", 0, 2), ("q", 0, 3), ("q", 1, 2), ("q", 1, 3),
            ]
        else:
            # phase 1: leftover V chunk halves, then Q quarters in deadline
            # order (each pair's qt completes before its unit starts)
            for kc in range(NVF, nkc):
                f_q[0] += [("v", kc, 0), ("v", kc, 1)]
            for p in range(2, NPAIR):
                f_q[0] += [("q", p, 0), ("q", p, 1)]
            f_q[0] += [("q", 0, 2), ("q", 0, 3), ("q", 1, 2), ("q", 1, 3)]
        # phase 2: Q half-1 quarters + O projection of query blocks 0-3;
        # qc4 partials (pairs 0-4) last so pair-7 finishers land in the tail
        # explicit order: each pair's Q quarters complete before its unit;
        # O sub-groups stay adjacent (proj-ring rule); the excess drains in
        # the post-loop flush
        f_q[1] = [
            ("q", 2, 2), ("q", 2, 3), ("q", 3, 2),
            ("q", 3, 3), ("os", 0, 0, 0), ("os", 0, 0, 1),
            ("q", 4, 2), ("q", 4, 3), ("os", 0, 1, 0),
            ("os", 0, 1, 1), ("q", 5, 2), ("q", 5, 3),
            ("os", 1, 0, 0), ("os", 1, 0, 1), ("q", 6, 2),
            ("q", 6, 3), ("q", 7, 2), ("q", 7, 3),
            ("os", 1, 1, 0), ("os", 1, 1, 1), ("os", 2, 0, 0),
            ("os", 2, 0, 1), ("os", 2, 1, 0), ("os", 2, 1, 1),
            ("os", 3, 0, 0), ("os", 3, 0, 1), ("os", 3, 1, 0),
            ("os", 3, 1, 1), ("opre", 4, 0), ("opre", 4, 1),
        ]
        oproj_done = {(0, 0), (0, 1), (1, 0), (1, 1), (2, 0), (2, 1), (3, 0), (3, 1)}

        def pop_filler(phase):
            if not f_q[phase]:
                return False
            u = f_q[phase].pop(0)
            if u[0] == "q":
                emit_qt_quarter(u[1], u[2])
            elif u[0] == "v":
                emit_vv(u[1], half=u[2])
            elif u[0] == "vs":
                emit_vv_sub(u[1], u[2], u[3])
            elif u[0] == "os":
                emit_oproj_sub(u[1], u[2], u[3])
            elif u[0] == "opre":
                # pairs 0-4 only: pair 5+'s hq1 ct is not yet written when
                # these pop near the end of phase 2
                emit_oproj_pre(u[1], u[2], 5)
            return True

        # ---- pre-phase (DMA-paced): K projections, first V chunks, the
        # first two Q halves. The DMA queue only starts delivering ~9us in
        # and ramps to ~300GB/s; this front consumes exactly at that pace.
        for p in range(NPAIR):
            emit_kt_half(p, 0)
            emit_kt_half(p, 1)
        for kc in range(NVF):
            emit_vv(kc)
        for p in range(nq_front):
            emit_qt_half(p, 0)

        # ---- attention: software-pipelined over (hq, pair, kc) ---------
        # The PE queue is in-order, and PV(kc) blocks on exp(kc); emitting
        # scores LA positions ahead of PV keeps completed score tiles
        # queued for the scalar engine so the exp stream never starves.
        units = [(hq, p) for hq in range(2) for p in range(NPAIR)]
        SPOS = [(ui, kc) for ui in range(len(units)) for kc in range(nkc)]
        LA = min(5, nkc) if AGG else min(4, nkc)
        pvs = {}
        sts = {}

        for p in range(NPAIR):
            ct_t[p] = big.tile([P, S], dt.bfloat16, tag="ct", name=f"ct{p}")

        def emit_sc(ui, kc):
            hq, p = units[ui]
            qs = slice(hq * 512, (hq + 1) * 512)
            st = psp.tile([P, 1024], dt.float32, tag="st")
            sts[(ui, kc)] = st
            ks = slice(kc * P, (kc + 1) * P)
            nc.tensor.matmul(
                st[:, 0:512],
                kt_t[p][0:DK, ks],
                qt_t[p][0:DK, qs],
                start=True,
                stop=True,
                tile_position=(0, 0),
            )
            nc.tensor.matmul(
                st[:, 512:1024],
                kt_t[p][DK:P, ks],
                qt_t[p][DK:P, qs],
                start=True,
                stop=True,
                tile_position=(DK, 0),
            )
            et = strm.tile([P, 1024], dt.bfloat16, tag="et", bufs=ET_BUFS)
            nc.scalar.activation(
                et[:], st[:], AF.Exp, bias=msk_sb[:, kc : kc + 1], scale=1.0
            )
            sts[(ui, kc)] = et

        def emit_pv(ui, kc):
            hq, p = units[ui]
            if kc == 0:
                pvs[ui] = (
                    psp.tile([P, 512], dt.float32, tag="pv", name=f"pv0_{ui}"),
                    psp.tile([P, 512], dt.float32, tag="pv", name=f"pv1_{ui}"),
                )
            pv0, pv1 = pvs[ui]
            et = sts.pop((ui, kc))
            for hloc, pv in ((0, pv0), (1, pv1)):
                nc.tensor.matmul(
                    pv[:],
                    vv_t[kc][:, (2 * p + hloc) * VW : (2 * p + hloc + 1) * VW],
                    et[:, hloc * 512 : (hloc + 1) * 512],
                    start=(kc == 0),
                    stop=(kc == nkc - 1),
                )

        def emit_norm(ui):
            hq, p = units[ui]
            qs = slice(hq * 512, (hq + 1) * 512)
            pv0, pv1 = pvs.pop(ui)
            for hloc, pv in ((0, pv0), (1, pv1)):
                # evacuate the whole pv tile (den at partition 0, dims at
                # 64..127) to SBUF in one DVE op -- this frees the PSUM
                # bank so the next unit's PV can start; the rest of the
                # softmax-normalize chain runs out of SBUF off that ring
                pvc = strm.tile([P, 512], dt.float32, tag="pvc", bufs=PC_BUFS)
                nc.vector.tensor_copy(pvc[:], pv[:])
                rcp = strm.tile([1, 512], dt.float32, tag="rcp", bufs=CH_BUFS)
                nc.vector.reciprocal_approx_fast(rcp[:], pvc[0:1, :])
                rb = strm.tile([P, 512], dt.float32, tag="rb", bufs=CH_BUFS)
                nc.gpsimd.partition_broadcast(rb[:], rcp[:])
                # phase-1 multiplies run on the (otherwise idle) gpsimd so
                # the DVE queue stays clear for the pvc copies that free
                # the pv PSUM ring at each unit boundary
                mul_eng = nc.gpsimd if hq == 0 else nc.vector
                mul_eng.tensor_mul(
                    ct_t[p][hloc * DK : (hloc + 1) * DK, qs],
                    pvc[64:P, :],
                    rb[64:P, :],
                )

        for i, (ui, kc) in enumerate(SPOS):
            emit_sc(ui, kc)
            hq = units[ui][0]
            if AGG and hq == 0:
                # phase-1 fillers pop before the trailing PV so V chunks
                # land just ahead of their first PV consumer
                pop_filler(0)
            j = i - LA
            if j >= 0:
                uj, kj = SPOS[j]
                emit_pv(uj, kj)
                if kj == nkc - 1:
                    emit_norm(uj)
            if hq == 1:
                if 1 <= kc <= 4:
                    pop_filler(1)
            elif not AGG and 1 <= kc <= 3:
                pop_filler(0)
        def drain_one(j):
            uj, kj = SPOS[j]
            emit_pv(uj, kj)
            if kj == nkc - 1:
                emit_norm(uj)

        if AGG:
            # The trailing PVs are ACT-paced (each waits its exp) leaving
            # PE gaps: slot the tail's O-projection prefixes into them.
            # Only the st ring frees mid-drain (as the last score tiles'
            # exps retire); the pv ring is held by unit 15 until its own
            # evacuation, so qc5's prefixes follow norm(15).
            for j in range(len(SPOS) - LA, len(SPOS) - 1):
                drain_one(j)
            st6 = psp.tile([P, 1024], dt.float32, tag="st", name="st6")
            for half in range(2):
                emit_oproj_pre(6, half, 5, ps=st6[:, half * 512 : (half + 1) * 512])
            drain_one(len(SPOS) - 1)
            st7 = psp.tile([P, 1024], dt.float32, tag="st", name="st7")
            for half in range(2):
                emit_oproj_pre(7, half, 5, ps=st7[:, half * 512 : (half + 1) * 512])
            for half in range(2):
                pvo = psp.tile([P, 512], dt.float32, tag="pv", name=f"pvo{half}")
                emit_oproj_pre(5, half, 5, ps=pvo)
        else:
            for j in range(len(SPOS) - LA, len(SPOS)):
                drain_one(j)

        # ---- output projection tail ------------------------------------
        # The final three normalize chains serialize on the DVE for ~6us
        # after the last exp. Cover that window with partial O-projection
        # accumulations (pairs 0-4, whose ct rows are long done) for every
        # remaining query block, using the now-idle pv and st PSUM rings as
        # extra accumulators; afterwards each block needs only a 3-matmul
        # finisher.
        while pop_filler(0) or pop_filler(1):
            pass
        if nkc >= 4:
            # (qc4 prefixes were queued as phase-2 fillers on the proj
            # ring; qc5-7 prefixes ran inside the drain on the AGG path)
            if not AGG:
                for half in range(2):
                    pvo = psp.tile(
                        [P, 512], dt.float32, tag="pv", name=f"pvo{half}"
                    )
                    emit_oproj_pre(5, half, 5, ps=pvo)
                st6 = psp.tile([P, 1024], dt.float32, tag="st", name="st6")
                for half in range(2):
                    emit_oproj_pre(
                        6, half, 5, ps=st6[:, half * 512 : (half + 1) * 512]
                    )
                st7 = psp.tile([P, 1024], dt.float32, tag="st", name="st7")
                for half in range(2):
                    emit_oproj_pre(
                        7, half, 5, ps=st7[:, half * 512 : (half + 1) * 512]
                    )
            for qc in range(4, NCH):
                for half in range(2):
                    emit_oproj_fin(qc, half, 5)
        else:
            for qc in range(NCH):
                for half in range(2):
                    if (qc, half) in o_ps:
                        emit_oproj_fin(qc, half, 5)
                    elif (qc, half) not in oproj_done:
                        emit_oproj(qc, half)

    nc.finalize()
    return nc


def _band(w: np.ndarray, ncol: int) -> np.ndarray:
    # w: [1024, ncol*128]. Output row-block p holds column-band p rearranged
    # as [128 rows (r), 8 chunks (di) x 128]: out[p*128+r, di*128+c] =
    # w[di*128+r, p*128+c]  -- the stationary layout for lhsT slices.
    return np.ascontiguousarray(
        w.reshape(NCH, P, ncol, P).transpose(2, 1, 0, 3).reshape(ncol * P, D)
    )


def _make_in_maps(query, key, value, mask, Wq, bq, Wk, bk, Wv, bv, Wo, bo):
    query = np.asarray(query, dtype=np.float32)
    key = np.asarray(key, dtype=np.float32)
    value = np.asarray(value, dtype=np.float32)
    mask = np.asarray(mask)
    Wq = np.asarray(Wq, dtype=np.float32)
    Wk = np.asarray(Wk, dtype=np.float32)
    Wv = np.asarray(Wv, dtype=np.float32)
    Wo = np.asarray(Wo, dtype=np.float32)
    sc = np.float32(1.0 / math.sqrt(DK))
    bo_eff = (np.asarray(bv, np.float32) @ Wo + np.asarray(bo, np.float32)).reshape(
        1, D
    )

    idxs, nv = [], []
    for i in range(B):
        ix = np.nonzero(np.asarray(mask[i, 0]) != 0)[0]
        idxs.append(ix)
        nv.append(len(ix))
    nkc = min(NCH, max(1, -(-max(nv) // P)))
    SK = nkc * P

    bf16 = ml_dtypes.bfloat16
    wqb = _band(Wq * sc, NCH).astype(bf16)
    wkb = _band(Wk, NCH).astype(bf16)
    wv_b = np.ascontiguousarray(Wv).astype(bf16)
    wo_b = np.ascontiguousarray(Wo).astype(bf16)
    bq2 = np.ascontiguousarray((np.asarray(bq, np.float32) * sc).reshape(NCH, P).T)
    bk2 = np.ascontiguousarray(np.asarray(bk, np.float32).reshape(NCH, P).T)

    in_maps = []
    for i in range(B):
        ix = idxs[i]
        pad = SK - len(ix)
        ixp = np.concatenate([ix, np.zeros(pad, dtype=ix.dtype)])
        mb = np.full(SK, 0.0, dtype=np.float32)
        if pad:
            mb[len(ix) :] = NEGB
        kTg = np.ascontiguousarray(key[i][ixp].astype(bf16).T)
        vT = value[i][ixp].astype(bf16).T  # [D, SK]
        vgb = np.ascontiguousarray(_band(vT, nkc))
        in_maps.append(
            {
                "qT": np.ascontiguousarray(query[i].astype(bf16).T),
                "kTg": kTg,
                "vgb": vgb,
                "wqb": wqb,
                "wkb": wkb,
                "wv": wv_b,
                "wo": wo_b,
                "bq": bq2,
                "bk": bk2,
                "msk": np.ascontiguousarray(mb.reshape(nkc, P).T),
                "bo": bo_eff,
            }
        )
    return nkc, in_maps


def kernel(query, key, value, mask, Wq, bq, Wk, bk, Wv, bv, Wo, bo):
    nkc, in_maps = _make_in_maps(
        query, key, value, mask, Wq, bq, Wk, bk, Wv, bv, Wo, bo
    )
    if nkc not in _NC_CACHE:
        _NC_CACHE[nkc] = build_nc(nkc)
    nc = _NC_CACHE[nkc]
    res = run_bass_kernel_spmd(nc, in_maps, list(range(B)))
    return np.stack([res.results[i]["out"] for i in range(B)], axis=0).astype(
        np.float32
    )


# revision 45
# speedup vs baseline: 1.9166x; 1.9166x over previous
# Multi-headed attention (B=8, S=1024, D=1024, H=16) on 8 TRN2 NeuronCores.
# Strategy: pure batch data-parallel (one batch element per core, no
# collectives), all matmuls bf16 with fp32 PSUM accumulation. ~192us HW
# exec (staged baseline: 197us; original naive kernel: 430us).
#
# Structure: a DMA-paced front (warmup + all K projections + the first
# four Q halves, consuming exactly at the 2-ring DMA delivery rate), then
# a software-pipelined attention phase over (query-half, head-pair, key-
# chunk) with scores emitted 5 positions ahead of PV so the scalar
# engine's exp stream starts ~45us in and stays fed; ALL V-projection
# work drains just-in-time from deadline-ordered filler queues inside the
# attention loop. A staged output-projection tail finishes the run.
#
# Key optimizations:
#   - masked key positions are dropped on the host: key/value are gathered
#     to the unmasked positions (padded to a multiple of 128; an exp bias
#     of -30000 zeroes the pads exactly, matching the reference's -1e9
#     mask). The program is compiled per padded chunk count (nkc), cached.
#   - weights are pre-banded on the host so every DMA is a plain
#     contiguous [128, N] block transfer, issued alternately on the two
#     HWDGE queues (SP + Activation) for ~2x front delivery bandwidth.
#   - V tiles carry a per-head [ones | 63 dead | 64 dims] 128-wide group:
#     the PV matmul then lands the softmax denominator on PSUM partition 0
#     (the extra stationary columns are free - matmul cost is set by the
#     moving free dim). The whole PV tile is evacuated to SBUF in one DVE
#     op (cost depends only on free size), freeing the PSUM ring; the
#     reciprocal/broadcast/multiply normalize chain runs out of SBUF off
#     the critical path.
#   - all remaining projection work (Q quarters, late V chunks, and the
#     phase-2 O-projection of query blocks 0-3 in 4-matmul sub-groups)
#     drains from deadline-ordered filler queues inside the attention
#     loop, keeping the PE at full clock (HAM) while exp runs.
#   - the O-projection tail pre-accumulates pairs 0-4 for query blocks
#     4-7 on the then-idle pv/st PSUM rings while the final normalize
#     chains drain, leaving only short finishers after the last pair.
#   - the output bias (bv @ Wo + bo) is added on-device during the output
#     projection evacuation; output is written back bf16 to halve the
#     final writeback DMA (rel err stays ~5e-3, well under the 2e-2 gate).
import math
import sys

sys.path.insert(0, "/opt/trn_rl_repo")

from contextlib import ExitStack

import ml_dtypes
import numpy as np

import concourse.bass as bass
import concourse.mybir as mybir
from concourse import bacc
from concourse import tile
from concourse.bass_utils import run_bass_kernel_spmd

dt = mybir.dt
AF = mybir.ActivationFunctionType

B, S, D, H, DK = 8, 1024, 1024, 16, 64
P = 128
NCH = D // P  # 8 chunks of 128 along the 1024-sized dims
NPAIR = H // 2  # 8 head pairs
NEGB = -30000.0  # exp underflows to exactly 0.0, matching the -1e9 masking

_NC_CACHE = {}


def build_nc(nkc: int):
    SK = nkc * P  # gathered+padded key length
    SK2 = SK // 2
    lean = nkc >= 7  # dense-mask fallback: shallower stream buffers
    ET_BUFS = 3 if lean else 7
    PC_BUFS = 2 if lean else 3
    OB_BUFS = 2 if lean else 3
    CH_BUFS = 1 if lean else 2
    nc = bacc.Bacc()
    qT = nc.dram_tensor("qT", [D, S], dt.bfloat16, kind="ExternalInput")
    kTg = nc.dram_tensor("kTg", [D, SK], dt.bfloat16, kind="ExternalInput")
    vgb = nc.dram_tensor("vgb", [SK, D], dt.bfloat16, kind="ExternalInput")
    wqb = nc.dram_tensor("wqb", [D, D], dt.bfloat16, kind="ExternalInput")
    wkb = nc.dram_tensor("wkb", [D, D], dt.bfloat16, kind="ExternalInput")
    wv = nc.dram_tensor("wv", [D, D], dt.bfloat16, kind="ExternalInput")
    wo = nc.dram_tensor("wo", [D, D], dt.bfloat16, kind="ExternalInput")
    bq = nc.dram_tensor("bq", [P, NCH], dt.float32, kind="ExternalInput")
    bk = nc.dram_tensor("bk", [P, NCH], dt.float32, kind="ExternalInput")
    msk = nc.dram_tensor("msk", [P, nkc], dt.float32, kind="ExternalInput")
    bo = nc.dram_tensor("bo", [1, D], dt.float32, kind="ExternalInput")
    out = nc.dram_tensor("out", [S, D], dt.bfloat16, kind="ExternalOutput")

    with tile.TileContext(nc) as tc, ExitStack() as ctx:
        big = ctx.enter_context(tc.tile_pool(name="big", bufs=NCH))
        vp = ctx.enter_context(tc.tile_pool(name="vp", bufs=nkc))
        strm = ctx.enter_context(tc.tile_pool(name="strm", bufs=4))
        one = ctx.enter_context(tc.tile_pool(name="one", bufs=1))
        psp = ctx.enter_context(tc.tile_pool(name="psp", bufs=2, space="PSUM"))

        # ---- DMA emission in consumption order -------------------------
        # alternate the two HWDGE queues (SP + Activation) so the front
        # loads stream on two rings in parallel
        _dq = [0]

        def dma_load(dst, srcv):
            eng = nc.sync if _dq[0] % 2 == 0 else nc.scalar
            _dq[0] += 1
            eng.dma_start(dst, srcv)

        # PE warmup on a zeroed scratch tile: keeps the HAM activity window
        # busy while the first DMAs land so real work starts at 2.4 GHz.
        scr = one.tile([P, 512], dt.bfloat16, tag="scr")
        nc.gpsimd.memset(scr[:], 0.0)
        wps = psp.tile([P, 512], dt.float32, tag="proj")
        for _ in range(80):
            nc.tensor.matmul(
                wps[:, 0:P], scr[:, 0:P], scr[:, 512 - P : 512], start=True, stop=True
            )

        wkb_sb = [None] * NPAIR

        def load_wkb(p):
            t = big.tile([P, D], dt.bfloat16, tag="wkb")
            dma_load(t[:], wkb[p * P : (p + 1) * P, :])
            wkb_sb[p] = t

        load_wkb(0)
        xk = []
        for di in range(NCH):
            t = big.tile([P, SK], dt.bfloat16, tag="xk")
            dma_load(t[:], kTg[di * P : (di + 1) * P, :])
            xk.append(t)
        for p in range(1, NPAIR):
            load_wkb(p)

        # small constants
        msk_sb = one.tile([P, nkc], dt.float32, tag="msk")
        nc.sync.dma_start(msk_sb[:], msk[:])
        bq_sb = one.tile([P, NCH], dt.float32, tag="bq")
        nc.sync.dma_start(bq_sb[:], bq[:])
        bk_sb = one.tile([P, NCH], dt.float32, tag="bk")
        nc.sync.dma_start(bk_sb[:], bk[:])
        bo_row = one.tile([1, D], dt.float32, tag="bo_row")
        nc.sync.dma_start(bo_row[:], bo[:])

        # warm the ACT exp table while DMAs stream
        warm = one.tile([1, nkc], dt.float32, tag="warm")
        nc.scalar.activation(warm[:], msk_sb[0:1, :], AF.Exp, bias=0.0, scale=1.0)

        # output-bias row broadcast to all partitions
        bo_sb = one.tile([P, D], dt.float32, tag="bo_sb")
        nc.gpsimd.partition_broadcast(bo_sb[:], bo_row[:])

        # remaining loads, in consumption order
        wqb_sb = [None] * NPAIR

        def load_wqb(p):
            t = big.tile([P, D], dt.bfloat16, tag="wqb")
            dma_load(t[:], wqb[p * P : (p + 1) * P, :])
            wqb_sb[p] = t

        vgb_sb = [None] * nkc

        def load_vgb(kc):
            t = vp.tile([P, D], dt.bfloat16, tag="vgb")
            dma_load(t[:], vgb[kc * P : (kc + 1) * P, :])
            vgb_sb[kc] = t

        # Aggressive schedule (the common nkc==5 case): the front carries
        # only the K projections + four Q halves (exactly the DMA ramp);
        # ALL V-projection work drains as attention fillers, so the exp
        # stream starts ~13us earlier.
        AGG = nkc == 5
        NVF = 0 if AGG else (nkc if nkc >= 6 else min(3, nkc))
        nq_front = 4 if AGG else 2
        for p in range(nq_front):
            load_wqb(p)
        xq = []
        for di in range(NCH):
            t = big.tile([P, S], dt.bfloat16, tag="xq")
            dma_load(t[:], qT[di * P : (di + 1) * P, :])
            xq.append(t)
        wv_sb = []
        for di in range(NCH):
            t = big.tile([P, D], dt.bfloat16, tag="wv")
            dma_load(t[:], wv[di * P : (di + 1) * P, :])
            wv_sb.append(t)
        for kc in range(nkc):
            load_vgb(kc)
        for p in range(nq_front, NPAIR):
            load_wqb(p)
        wo_sb = []
        for pc in range(NCH):
            t = big.tile([P, D], dt.bfloat16, tag="wo")
            dma_load(t[:], wo[pc * P : (pc + 1) * P, :])
            wo_sb.append(t)

        # ---- work-unit emitters ---------------------------------------
        kt_t = [None] * NPAIR

        def emit_kt_half(p, half):
            # K projection of pair p, key half `half`, [d, s_k] layout
            if kt_t[p] is None:
                kt_t[p] = big.tile([P, SK], dt.bfloat16, tag="kt", name=f"kt{p}")
            hs = slice(half * SK2, (half + 1) * SK2)
            ps = psp.tile([P, 512], dt.float32, tag="proj", name=f"kt_ps{p}_{half}")
            for di in range(NCH):
                nc.tensor.matmul(
                    ps[:, 0:SK2],
                    wkb_sb[p][:, di * P : (di + 1) * P],
                    xk[di][:, hs],
                    start=(di == 0),
                    stop=(di == NCH - 1),
                )
            nc.vector.tensor_scalar_add(
                kt_t[p][:, hs], ps[:, 0:SK2], bk_sb[:, p : p + 1]
            )

        # V tiles: per head a 128-wide group [ones | 63 dead | 64 dims] so
        # the PV output puts the softmax denominator on PSUM partition 0
        # (readable in place by reciprocal_approx_fast) and the dims at
        # partitions 64..127 (PSUM partition ranges cannot straddle the 64
        # line). The extra M is free: matmul cost is set by the moving free
        # dim N, not M.
        VW = P
        vv_t = [None] * nkc

        v_ps = {}

        def emit_vv_sub(kc, hf, sub):
            # V projection chunk kc, head-half hf, contraction sub-range
            # (di 0-3 / 4-7); the psum evacuation rides on sub 1
            if vv_t[kc] is None:
                t = vp.tile([P, H * VW], dt.bfloat16, tag="vv", name=f"vv{kc}")
                vv_t[kc] = t
                nc.gpsimd.memset(t[:], 1.0)
            t = vv_t[kc]
            hs = slice(hf * 512, (hf + 1) * 512)
            if sub == 0:
                v_ps[(kc, hf)] = psp.tile(
                    [P, 512], dt.float32, tag="proj", name=f"v_ps{kc}_{hf}"
                )
            ps = v_ps[(kc, hf)]
            for di in range(4 * sub, 4 * sub + 4):
                nc.tensor.matmul(
                    ps[:],
                    vgb_sb[kc][:, di * P : (di + 1) * P],
                    wv_sb[di][:, hs],
                    start=(di == 0),
                    stop=(di == NCH - 1),
                )
            if sub == 1:
                del v_ps[(kc, hf)]
                dst = t[:, hf * 8 * VW : (hf + 1) * 8 * VW].rearrange(
                    "p (h e) -> p h e", e=VW
                )[:, :, 64 : 64 + DK]
                srcv = ps[:].rearrange("p (h e) -> p h e", e=DK)
                nc.vector.tensor_copy(dst, srcv)

        def emit_vv(kc, half=None):
            halves = (0, 1) if half is None else (half,)
            for hf in halves:
                emit_vv_sub(kc, hf, 0)
                emit_vv_sub(kc, hf, 1)

        qt_t = [None] * NPAIR

        def emit_qt_quarter(p, quarter):
            if qt_t[p] is None:
                qt_t[p] = big.tile([P, S], dt.bfloat16, tag="qt", name=f"qt{p}")
            qs = slice(quarter * 256, (quarter + 1) * 256)
            ps = psp.tile([P, 512], dt.float32, tag="proj", name=f"qt_ps{p}_{quarter}")
            for di in range(NCH):
                nc.tensor.matmul(
                    ps[:, 0:256],
                    wqb_sb[p][:, di * P : (di + 1) * P],
                    xq[di][:, qs],
                    start=(di == 0),
                    stop=(di == NCH - 1),
                )
            nc.vector.tensor_scalar_add(
                qt_t[p][:, qs], ps[:, 0:256], bq_sb[:, p : p + 1]
            )

        def emit_qt_half(p, hq):
            emit_qt_quarter(p, 2 * hq)
            emit_qt_quarter(p, 2 * hq + 1)

        ct_t = [None] * NPAIR
        o_ps = {}

        def emit_oproj_pre(qc, half, npc, ps=None):
            # output projection for query block qc, D-half `half`: partial
            # accumulation over pairs 0..npc-1 (their ct rows are ready
            # before the last pair's)
            hs = slice(half * 512, (half + 1) * 512)
            if ps is None:
                ps = psp.tile(
                    [P, 512], dt.float32, tag="proj", name=f"o_ps{qc}_{half}"
                )
            o_ps[(qc, half)] = ps
            for pc in range(npc):
                nc.tensor.matmul(
                    ps[:],
                    ct_t[pc][:, qc * P : (qc + 1) * P],
                    wo_sb[pc][:, hs],
                    start=(pc == 0),
                    stop=False,
                )

        def emit_oproj_ext(qc, half, a, b):
            # extend a partial O-projection accumulation over pairs a..b-1
            hs = slice(half * 512, (half + 1) * 512)
            ps = o_ps[(qc, half)]
            for pc in range(a, b):
                nc.tensor.matmul(
                    ps[:],
                    ct_t[pc][:, qc * P : (qc + 1) * P],
                    wo_sb[pc][:, hs],
                    start=False,
                    stop=False,
                )

        def emit_oproj_fin(qc, half, npc):
            # remaining pairs + bias + writeback
            hs = slice(half * 512, (half + 1) * 512)
            ps = o_ps.pop((qc, half))
            for pc in range(npc, NCH):
                nc.tensor.matmul(
                    ps[:],
                    ct_t[pc][:, qc * P : (qc + 1) * P],
                    wo_sb[pc][:, hs],
                    start=False,
                    stop=(pc == NCH - 1),
                )
            ob = strm.tile([P, 512], dt.bfloat16, tag="ob", bufs=OB_BUFS)
            nc.vector.tensor_add(ob[:], ps[:], bo_sb[:, hs])
            nc.sync.dma_start(out[qc * P : (qc + 1) * P, hs], ob[:])

        def emit_oproj_sub(qc, half, sub):
            hs = slice(half * 512, (half + 1) * 512)
            if sub == 0:
                o_ps[(qc, half)] = psp.tile(
                    [P, 512], dt.float32, tag="proj", name=f"o_ps{qc}_{half}"
                )
            ps = o_ps[(qc, half)]
            for pc in range(4 * sub, 4 * sub + 4):
                nc.tensor.matmul(
                    ps[:],
                    ct_t[pc][:, qc * P : (qc + 1) * P],
                    wo_sb[pc][:, hs],
                    start=(pc == 0),
                    stop=(pc == NCH - 1),
                )
            if sub == 1:
                del o_ps[(qc, half)]
                ob = strm.tile([P, 512], dt.bfloat16, tag="ob", bufs=OB_BUFS)
                nc.vector.tensor_add(ob[:], ps[:], bo_sb[:, hs])
                nc.sync.dma_start(out[qc * P : (qc + 1) * P, hs], ob[:])

        def emit_oproj(qc, half):
            emit_oproj_sub(qc, half, 0)
            emit_oproj_sub(qc, half, 1)

        def emit_dummies(n, name):
            dps = psp.tile([P, 512], dt.float32, tag="proj", name=name)
            for _ in range(n):
                nc.tensor.matmul(
                    dps[:, 0:P],
                    scr[:, 0:P],
                    scr[:, 512 - P : 512],
                    start=True,
                    stop=True,
                )

        # ---- filler queues --------------------------------------------
        # Attention is ACT(exp)-bound: each (pair, query-half) unit costs
        # ~5.7us of exp on the scalar engine vs ~3.2us of scores+PV on the
        # PE. The leftover PE capacity runs "filler" units popped from a
        # queue: remaining V chunks, Q projection quarters, and in phase 2
        # the O projection of query blocks 0-3 (whose ct rows completed in
        # phase 1). Units are small (~0.9-1.8us) so the 2-deep st ring can
        # absorb the delay they add before the next scores issue.
        f_q = [[], []]
        if AGG:
            # deadline-ordered: all V chunk halves (heads-half 0 feeds
            # pairs 0-3 almost immediately, half 1 feeds pair 4+), the
            # remaining Q-half-0 quarters, then the first Q-half-1s.
            # Units 0-1 pop 5 slots (kc 0-4) BEFORE the trailing PV so
            # each V chunk lands just ahead of its first PV use (LA=5).
            for kc in range(nkc):
                f_q[0] += [("vs", kc, 0, 0), ("vs", kc, 0, 1)]
            f_q[0] += [("q", 4, 0), ("q", 4, 1), ("q", 5, 0), ("q", 5, 1)]
            for kc in range(nkc):
                f_q[0] += [("vs", kc, 1, 0), ("vs", kc, 1, 1)]
            f_q[0] += [
                ("q", 6, 0), ("q", 6, 1), ("q", 7, 0), ("q", 7, 1),
                ("q", 0, 2), ("q", 0, 3), ("q", 1, 2), ("q", 1, 3),
            ]
        else:
            # phase 1: leftover V chunk halves, then Q quarters in deadline
            # order (each pair's qt completes before its unit starts)
            for kc in range(NVF, nkc):
                f_q[0] += [("v", kc, 0), ("v", kc, 1)]
            for p in range(2, NPAIR):
                f_q[0] += [("q", p, 0), ("q", p, 1)]
            f_q[0] += [("q", 0, 2), ("q", 0, 3), ("q", 1, 2), ("q", 1, 3)]
        # phase 2: Q half-1 quarters + O projection of query blocks 0-3;
        # qc4 partials (pairs 0-4) last so pair-7 finishers land in the tail
        # explicit order: each pair's Q quarters complete before its unit;
        # O sub-groups stay adjacent (proj-ring rule); the excess drains in
        # the post-loop flush
        f_q[1] = [
            ("q", 2, 2), ("q", 2, 3), ("q", 3, 2),
            ("q", 3, 3), ("os", 0, 0, 0), ("os", 0, 0, 1),
            ("q", 4, 2), ("q", 4, 3), ("os", 0, 1, 0),
            ("os", 0, 1, 1), ("q", 5, 2), ("q", 5, 3),
            ("os", 1, 0, 0), ("os", 1, 0, 1), ("q", 6, 2),
            ("q", 6, 3), ("q", 7, 2), ("q", 7, 3),
            ("os", 1, 1, 0), ("os", 1, 1, 1), ("os", 2, 0, 0),
            ("os", 2, 0, 1), ("os", 2, 1, 0), ("os", 2, 1, 1),
            ("os", 3, 0, 0), ("os", 3, 0, 1), ("os", 3, 1, 0),
            ("os", 3, 1, 1), ("opre", 4, 0), ("opre", 4, 1),
        ]
        oproj_done = {(0, 0), (0, 1), (1, 0), (1, 1), (2, 0), (2, 1), (3, 0), (3, 1)}

        def pop_filler(phase):
            if not f_q[phase]:
                return False
            u = f_q[phase].pop(0)
            if u[0] == "q":
                emit_qt_quarter(u[1], u[2])
            elif u[0] == "v":
                emit_vv(u[1], half=u[2])
            elif u[0] == "vs":
                emit_vv_sub(u[1], u[2], u[3])
            elif u[0] == "os":
                emit_oproj_sub(u[1], u[2], u[3])
            elif u[0] == "opre":
                # pairs 0-4 only: pair 5+'s hq1 ct is not yet written when
                # these pop near the end of phase 2
                emit_oproj_pre(u[1], u[2], 5)
            return True

        # ---- pre-phase (DMA-paced): K projections, first V chunks, the
        # first two Q halves. The DMA queue only starts delivering ~9us in
        # and ramps to ~300GB/s; this front consumes exactly at that pace.
        for p in range(NPAIR):
            emit_kt_half(p, 0)
            emit_kt_half(p, 1)
        for kc in range(NVF):
            emit_vv(kc)
        for p in range(nq_front):
            emit_qt_half(p, 0)

        # ---- attention: software-pipelined over (hq, pair, kc) ---------
        # The PE queue is in-order, and PV(kc) blocks on exp(kc); emitting
        # scores LA positions ahead of PV keeps completed score tiles
        # queued for the scalar engine so the exp stream never starves.
        units = [(hq, p) for hq in range(2) for p in range(NPAIR)]
        SPOS = [(ui, kc) for ui in range(len(units)) for kc in range(nkc)]
        LA = min(5, nkc) if AGG else min(4, nkc)
        pvs = {}
        sts = {}

        for p in range(NPAIR):
            ct_t[p] = big.tile([P, S], dt.bfloat16, tag="ct", name=f"ct{p}")

        def emit_sc(ui, kc):
            hq, p = units[ui]
            qs = slice(hq * 512, (hq + 1) * 512)
            st = psp.tile([P, 1024], dt.float32, tag="st")
            sts[(ui, kc)] = st
            ks = slice(kc * P, (kc + 1) * P)
            nc.tensor.matmul(
                st[:, 0:512],
                kt_t[p][0:DK, ks],
                qt_t[p][0:DK, qs],
                start=True,
                stop=True,
                tile_position=(0, 0),
            )
            nc.tensor.matmul(
                st[:, 512:1024],
                kt_t[p][DK:P, ks],
                qt_t[p][DK:P, qs],
                start=True,
                stop=True,
                tile_position=(DK, 0),
            )
            et = strm.tile([P, 1024], dt.bfloat16, tag="et", bufs=ET_BUFS)
            nc.scalar.activation(
                et[:], st[:], AF.Exp, bias=msk_sb[:, kc : kc + 1], scale=1.0
            )
            sts[(ui, kc)] = et

        def emit_pv(ui, kc):
            hq, p = units[ui]
            if kc == 0:
                pvs[ui] = (
                    psp.tile([P, 512], dt.float32, tag="pv", name=f"pv0_{ui}"),
                    psp.tile([P, 512], dt.float32, tag="pv", name=f"pv1_{ui}"),
                )
            pv0, pv1 = pvs[ui]
            et = sts.pop((ui, kc))
            for hloc, pv in ((0, pv0), (1, pv1)):
                nc.tensor.matmul(
                    pv[:],
                    vv_t[kc][:, (2 * p + hloc) * VW : (2 * p + hloc + 1) * VW],
                    et[:, hloc * 512 : (hloc + 1) * 512],
                    start=(kc == 0),
                    stop=(kc == nkc - 1),
                )

        def emit_norm(ui):
            hq, p = units[ui]
            qs = slice(hq * 512, (hq + 1) * 512)
            pv0, pv1 = pvs.pop(ui)
            for hloc, pv in ((0, pv0), (1, pv1)):
                # evacuate the whole pv tile (den at partition 0, dims at
                # 64..127) to SBUF in one DVE op -- this frees the PSUM
                # bank so the next unit's PV can start; the rest of the
                # softmax-normalize chain runs out of SBUF off that ring
                pvc = strm.tile([P, 512], dt.float32, tag="pvc", bufs=PC_BUFS)
                nc.vector.tensor_copy(pvc[:], pv[:])
                rcp = strm.tile([1, 512], dt.float32, tag="rcp", bufs=CH_BUFS)
                nc.vector.reciprocal_approx_fast(rcp[:], pvc[0:1, :])
                rb = strm.tile([P, 512], dt.float32, tag="rb", bufs=CH_BUFS)
                nc.gpsimd.partition_broadcast(rb[:], rcp[:])
                nc.vector.tensor_mul(
                    ct_t[p][hloc * DK : (hloc + 1) * DK, qs],
                    pvc[64:P, :],
                    rb[64:P, :],
                )

        for i, (ui, kc) in enumerate(SPOS):
            emit_sc(ui, kc)
            hq = units[ui][0]
            if AGG and hq == 0:
                # phase-1 fillers pop before the trailing PV so V chunks
                # land just ahead of their first PV consumer
                pop_filler(0)
            j = i - LA
            if j >= 0:
                uj, kj = SPOS[j]
                emit_pv(uj, kj)
                if kj == nkc - 1:
                    emit_norm(uj)
            if hq == 1:
                if 1 <= kc <= 4:
                    pop_filler(1)
            elif not AGG and 1 <= kc <= 3:
                pop_filler(0)
        def drain_one(j):
            uj, kj = SPOS[j]
            emit_pv(uj, kj)
            if kj == nkc - 1:
                emit_norm(uj)

        if AGG:
            # The trailing PVs are ACT-paced (each waits its exp) leaving
            # PE gaps: slot the tail's O-projection prefixes into them.
            # Only the st ring frees mid-drain (as the last score tiles'
            # exps retire); the pv ring is held by unit 15 until its own
            # evacuation, so qc5's prefixes follow norm(15).
            for j in range(len(SPOS) - LA, len(SPOS) - 1):
                drain_one(j)
            st6 = psp.tile([P, 1024], dt.float32, tag="st", name="st6")
            for half in range(2):
                emit_oproj_pre(6, half, 5, ps=st6[:, half * 512 : (half + 1) * 512])
            drain_one(len(SPOS) - 1)
            st7 = psp.tile([P, 1024], dt.float32, tag="st", name="st7")
            for half in range(2):
                emit_oproj_pre(7, half, 5, ps=st7[:, half * 512 : (half + 1) * 512])
            for half in range(2):
                pvo = psp.tile([P, 512], dt.float32, tag="pv", name=f"pvo{half}")
                emit_oproj_pre(5, half, 5, ps=pvo)
        else:
            for j in range(len(SPOS) - LA, len(SPOS)):
                drain_one(j)

        # ---- output projection tail ------------------------------------
        # The final three normalize chains serialize on the DVE for ~6us
        # after the last exp. Cover that window with partial O-projection
        # accumulations (pairs 0-4, whose ct rows are long done) for every
        # remaining query block, using the now-idle pv and st PSUM rings as
        # extra accumulators; afterwards each block needs only a 3-matmul
        # finisher.
        while pop_filler(0) or pop_filler(1):
            pass
        if nkc >= 4:
            # (qc4 prefixes were queued as phase-2 fillers on the proj
            # ring; qc5-7 prefixes ran inside the drain on the AGG path)
            if not AGG:
                for half in range(2):
                    pvo = psp.tile(
                        [P, 512], dt.float32, tag="pv", name=f"pvo{half}"
                    )
                    emit_oproj_pre(5, half, 5, ps=pvo)
                st6 = psp.tile([P, 1024], dt.float32, tag="st", name="st6")
                for half in range(2):
                    emit_oproj_pre(
                        6, half, 5, ps=st6[:, half * 512 : (half + 1) * 512]
                    )
                st7 = psp.tile([P, 1024], dt.float32, tag="st", name="st7")
                for half in range(2):
                    emit_oproj_pre(
                        7, half, 5, ps=st7[:, half * 512 : (half + 1) * 512]
                    )
            for qc in range(4, NCH):
                for half in range(2):
                    emit_oproj_ext(qc, half, 5, 7)
            for qc in range(4, NCH):
                for half in range(2):
                    emit_oproj_fin(qc, half, 7)
        else:
            for qc in range(NCH):
                for half in range(2):
                    if (qc, half) in o_ps:
                        emit_oproj_fin(qc, half, 5)
                    elif (qc, half) not in oproj_done:
                        emit_oproj(qc, half)

    nc.finalize()
    return nc


def _band(w: np.ndarray, ncol: int) -> np.ndarray:
    # w: [1024, ncol*128]. Output row-block p holds column-band p rearranged
    # as [128 rows (r), 8 chunks (di) x 128]: out[p*128+r, di*128+c] =
    # w[di*128+r, p*128+c]  -- the stationary layout for lhsT slices.
    return np.ascontiguousarray(
        w.reshape(NCH, P, ncol, P).transpose(2, 1, 0, 3).reshape(ncol * P, D)
    )


def _make_in_maps(query, key, value, mask, Wq, bq, Wk, bk, Wv, bv, Wo, bo):
    query = np.asarray(query, dtype=np.float32)
    key = np.asarray(key, dtype=np.float32)
    value = np.asarray(value, dtype=np.float32)
    mask = np.asarray(mask)
    Wq = np.asarray(Wq, dtype=np.float32)
    Wk = np.asarray(Wk, dtype=np.float32)
    Wv = np.asarray(Wv, dtype=np.float32)
    Wo = np.asarray(Wo, dtype=np.float32)
    sc = np.float32(1.0 / math.sqrt(DK))
    bo_eff = (np.asarray(bv, np.float32) @ Wo + np.asarray(bo, np.float32)).reshape(
        1, D
    )

    idxs, nv = [], []
    for i in range(B):
        ix = np.nonzero(np.asarray(mask[i, 0]) != 0)[0]
        idxs.append(ix)
        nv.append(len(ix))
    nkc = min(NCH, max(1, -(-max(nv) // P)))
    SK = nkc * P

    bf16 = ml_dtypes.bfloat16
    wqb = _band(Wq * sc, NCH).astype(bf16)
    wkb = _band(Wk, NCH).astype(bf16)
    wv_b = np.ascontiguousarray(Wv).astype(bf16)
    wo_b = np.ascontiguousarray(Wo).astype(bf16)
    bq2 = np.ascontiguousarray((np.asarray(bq, np.float32) * sc).reshape(NCH, P).T)
    bk2 = np.ascontiguousarray(np.asarray(bk, np.float32).reshape(NCH, P).T)

    in_maps = []
    for i in range(B):
        ix = idxs[i]
        pad = SK - len(ix)
        ixp = np.concatenate([ix, np.zeros(pad, dtype=ix.dtype)])
        mb = np.full(SK, 0.0, dtype=np.float32)
        if pad:
            mb[len(ix) :] = NEGB
        kTg = np.ascontiguousarray(key[i][ixp].astype(bf16).T)
        vT = value[i][ixp].astype(bf16).T  # [D, SK]
        vgb = np.ascontiguousarray(_band(vT, nkc))
        in_maps.append(
            {
                "qT": np.ascontiguousarray(query[i].astype(bf16).T),
                "kTg": kTg,
                "vgb": vgb,
                "wqb": wqb,
                "wkb": wkb,
                "wv": wv_b,
                "wo": wo_b,
                "bq": bq2,
                "bk": bk2,
                "msk": np.ascontiguousarray(mb.reshape(nkc, P).T),
                "bo": bo_eff,
            }
        )
    return nkc, in_maps


def kernel(query, key, value, mask, Wq, bq, Wk, bk, Wv, bv, Wo, bo):
    nkc, in_maps = _make_in_maps(
        query, key, value, mask, Wq, bq, Wk, bk, Wv, bv, Wo, bo
    )
    if nkc not in _NC_CACHE:
        _NC_CACHE[nkc] = build_nc(nkc)
    nc = _NC_CACHE[nkc]
    res = run_bass_kernel_spmd(nc, in_maps, list(range(B)))
    return np.stack([res.results[i]["out"] for i in range(B)], axis=0).astype(
        np.float32
    )
